# revision 14
# baseline (speedup 1.0000x reference)
"""Trainium2 Bass kernel for ConvexLinearAttention (elu(x)+1 linear attention).

Full-input contract: kernel(**inputs) takes the unsharded tensors
(x [2,2048,1024], wq/wk/wv/wo [1024,1024], bq/bk/bv/bo [1024]) and returns the
full output [2,2048,1024].

Sharding (8 cores): data-parallel over batch (2) x head-group-parallel (4 groups
of 4 heads).  Each core projects only its 256-wide head slice, runs the
linearized attention (attended = qf @ (kf^T V) / (qf @ sum(kf)) -- an exact
refactoring of the dense normalized scores), and emits a partial output
projection in natural [s, e] layout.  The host sums the 4 head-group partials
per batch.

All matmul operands are bf16 (fp32 PSUM accumulation): x/weights/features/
outputs stream as bf16, halving HBM traffic vs fp32.  KV^T is accumulated
directly (operand-swapped matmul) so the fused M = masked(KV) @ wo needs no
on-chip transpose; ksum rides as two columns of one extra PSUM bank via N=1
matmuls against a constant ones column (single accumulation group across both
head-groups -- PSUM start pending-zeroes a whole 2KB bank region, so the bank
must host exactly one group).  The attention denominator uses
reciprocal_approx_fast (~5x faster than the exact DVE reciprocal, ~18 correct
bits vs the 2e-2 tolerance).  Denominator matmuls are emitted one chunk ahead
of the out-projection so the PE never waits on the DVE and the HAM clock gate
stays open through phase B.
"""

from contextlib import ExitStack

import numpy as np
import ml_dtypes

import concourse.bass as bass
import concourse.mybir as mybir
import concourse.tile as tile
from concourse import bacc, bass_utils

F32 = mybir.dt.float32
BF16 = mybir.dt.bfloat16
AF = mybir.ActivationFunctionType
ALU = mybir.AluOpType

NPBF = ml_dtypes.bfloat16

D = 1024          # model dim
S = 2048          # sequence length
BATCH = 2
CSL = 256         # head-slice width per core (4 heads x 64)
NG = 2            # 128-wide c-groups per core
P = 128
NST = S // P      # 16 s-tiles
SC = 512          # s-chunk
NSC = S // SC     # 4 s-chunks
STC = SC // P     # 4 s-tiles per chunk
EH = 512          # e-half width for out-proj rhs

_CACHE: dict = {}


def install_ntff_hook_shim():
    """Provide ``antenv.axon_hooks`` when the image ships only the antenv stub.

    concourse.bass_utils imports it unconditionally on the axon trace path;
    without this shim trace=True (or BASS_TRACE=1) crashes.  Registers the real
    ctypes NTFF hook when the axon .so is present, else a None-returning stub
    so tracing degrades gracefully.
    """
    import os
    import sys
    import types

    if "antenv.axon_hooks" in sys.modules:
        return
    try:
        import antenv
        import antenv.axon_hooks  # noqa: F401
        return  # real module exists
    except ImportError:
        pass
    mod = types.ModuleType("antenv.axon_hooks")
    state: dict = {"h": None}
    mod.set_axon_ntff_profile_hook = lambda h: state.__setitem__("h", h)
    mod.get_axon_ntff_profile_hook = lambda: state.get("h")
    sys.modules["antenv.axon_hooks"] = mod
    antenv.axon_hooks = mod
    so_path = "/opt/axon/libaxon_pjrt.so"
    if os.path.exists(so_path):
        try:
            from trn_agent_boot.trn_boot import _ntff_profile_via_ctypes

            state["h"] = _ntff_profile_via_ctypes(so_path)
        except Exception:
            pass


def _build_kernel_body(ctx: ExitStack, tc: tile.TileContext, t, use_biases):
    nc = tc.nc
    # with biases, a 9th d-tile (ones row 0, zeros elsewhere) multiplies the
    # bias row appended to the weight matrices: exact bias add inside the GEMM
    ndt = 9 if use_biases else 8

    # host-prepared layouts keep every DMA descriptor contiguous per
    # partition (2-8KB runs): xA = first chunk per s-tile, xB = chunks 1-3
    xA = t["xA"].ap().rearrange("(st p) (do s) -> p st do s", p=P, do=8)
    xB = t["xB"].ap().rearrange("(sc p) (do s) -> p sc do s", p=P, do=8)
    wqT = t["wqT"].ap().rearrange("p (do c) -> p do c", do=ndt)
    wkvT = t["wkvT"].ap().rearrange("p (do c) -> p do c", do=ndt)
    woT = t["woT"].ap().rearrange("p (g e) -> p g e", g=NG)
    out2 = t["out2"].ap().rearrange("(st p) e -> p st e", p=P)

    const = ctx.enter_context(tc.tile_pool(name="const", bufs=1))

    def single(shape, name, dtype=BF16):
        return const.tile(shape, dtype, name=name, tag=name)

    wkv_sb = single([P, ndt, 2 * CSL], "wkv_sb")
    wq_sb = single([P, ndt, CSL], "wq_sb")
    wo_sb = single([P, NG, D], "wo_sb")
    x_sb = single([P, ndt, S], "x_sb")
    qf_sb = single([P, NG, S], "qf_sb")
    ones_sb = single([P, 1], "ones_sb")
    bkvT_sb = single([P, NG, P], "bkvT_sb")
    bden_sb = single([P, NG, P], "bden_sb")
    m_sb = single([P, NG, D], "m_sb")

    # ---- input DMA: the sync queue starts transfers earliest, so it
    # carries the whole critical path in need order (wkv, then x).  The
    # scalar/gpsimd queues (slower to start) carry weights needed later.
    nc.sync.dma_start(out=wkv_sb, in_=wkvT)
    for sti in range(STC):
        ssl = slice(sti * P, (sti + 1) * P)
        nc.sync.dma_start(out=x_sb[:, 0:8, ssl], in_=xA[:, sti, :, :])
    for sc in range(1, NSC):
        csl = slice(sc * SC, (sc + 1) * SC)
        nc.sync.dma_start(out=x_sb[:, 0:8, csl], in_=xB[:, sc - 1, :, :])
    # vector: tiny const needed by the first ksum matmul (~13us in)
    nc.vector.memset(ones_sb, 1.0)
    if use_biases:
        nc.vector.memset(x_sb[0:1, 8, :], 1.0)
        nc.vector.memset(x_sb[1:P, 8, :], 0.0)
    nc.scalar.dma_start(out=wq_sb, in_=wqT)
    nc.gpsimd.dma_start(out=wo_sb, in_=woT)
    nc.gpsimd.memset(bkvT_sb, 0.0)
    nc.gpsimd.memset(bden_sb, 0.0)

    # ---- phase A: K|V projection + feature map + KV^T/ksum accumulation,
    #      Q projection interleaved per s-chunk ----------------------------
    with tc.tile_pool(name="ps_kv", bufs=1, space="PSUM") as ps_kv:
        _phase_a(tc, nc, ps_kv, ndt, x_sb, wkv_sb, wq_sb, wo_sb, qf_sb,
                 ones_sb, bkvT_sb, bden_sb, m_sb)

    # ---- phase B: normalize q, out[s,e] = sum_g qs_g^T M_g ---------------
    # all den/recip/qs first (they only gate on ksum + qf), then the
    # out-projection streams PE-dense with copies pipelining behind it
    with tc.tile_pool(name="ps_d", bufs=1, space="PSUM") as ps_d, \
         tc.tile_pool(name="ps_o", bufs=3, space="PSUM") as ps_o, \
         tc.tile_pool(name="sb_qs", bufs=4) as sb_qs, \
         tc.tile_pool(name="sb_b", bufs=2) as sb_b:
        qs_tiles = []
        for sc in range(NSC):
            csl = slice(sc * SC, (sc + 1) * SC)
            qs_pair = []
            for g in range(NG):
                d_ps = ps_d.tile([P, SC], F32, tag=f"d_ps{g}")
                nc.tensor.matmul(d_ps, bden_sb[:, g, :], qf_sb[:, g, csl],
                                 start=True, stop=True)
                rden = sb_b.tile([P, SC], F32, tag=f"rden{g}")
                nc.vector.reciprocal_approx_fast(out=rden, in_=d_ps)
                # both operands SBUF => gpsimd can take it (no PSUM port)
                qs = sb_qs.tile([P, SC], BF16, tag=f"qs{g}")
                nc.gpsimd.tensor_tensor(qs, qf_sb[:, g, csl], rden, ALU.mult)
                qs_pair.append(qs)
            qs_tiles.append(qs_pair)
        for sc in range(NSC):
            _emit_outproj(nc, sb_b, ps_o, m_sb, out2, qs_tiles[sc], sc)


def _phase_a(tc, nc, ps_kv, ndt, x_sb, wkv_sb, wq_sb, wo_sb, qf_sb,
             ones_sb, bkvT_sb, bden_sb, m_sb):
    kvt_ps = [ps_kv.tile([P, P], F32, name=f"kvt_ps{g}", tag=f"kvt{g}")
              for g in range(NG)]
    # ksum columns for both groups share one bank => ONE accumulation group
    ksum_ps = ps_kv.tile([P, NG], F32, name="ksum_ps", tag="ksum")

    with tc.tile_pool(name="ps_a", bufs=3, space="PSUM") as ps_a, \
         tc.tile_pool(name="ps_q", bufs=2, space="PSUM") as ps_q, \
         tc.tile_pool(name="sb_a", bufs=3) as sb_a:

        def kv_chunk(sc):
            for sti in range(STC):
                st = sc * STC + sti
                ssl = slice(st * P, (st + 1) * P)
                # combined K|V projection: [s, 0:256]=K, [s, 256:512]=V
                kvp = ps_a.tile([P, 2 * CSL], F32, tag="kvp")
                for dt in range(ndt):
                    nc.tensor.matmul(
                        kvp, x_sb[:, dt, ssl], wkv_sb[:, dt, :],
                        start=(dt == 0), stop=(dt == ndt - 1))
                # kf = relu(K) + exp(min(K, 0))   (= elu(K)+1)
                kf = sb_a.tile([P, CSL], BF16, tag="kf")
                m_k = sb_a.tile([P, CSL], BF16, tag="m_k")
                nc.vector.tensor_scalar(m_k, kvp[:, 0:CSL], 0.0, None,
                                        op0=ALU.min)
                nc.scalar.activation(m_k, m_k, AF.Exp)
                nc.vector.scalar_tensor_tensor(
                    kf, kvp[:, 0:CSL], 0.0, m_k, op0=ALU.max, op1=ALU.add)
                v_sb = sb_a.tile([P, CSL], BF16, tag="v_sb")
                nc.scalar.copy(out=v_sb, in_=kvp[:, CSL:2 * CSL])

                # KV^T / ksum accumulation per 128-group:
                #   kvt[cv, ck] += v[s, cv]^T kf[s, ck];  ksum[ck] += kf^T 1
                for g in range(NG):
                    gsl = slice(g * P, (g + 1) * P)
                    nc.tensor.matmul(
                        kvt_ps[g], v_sb[:, gsl], kf[:, gsl],
                        start=(st == 0), stop=(st == NST - 1))
                    nc.tensor.matmul(
                        ksum_ps[:, g:g + 1], kf[:, gsl], ones_sb,
                        start=(st == 0 and g == 0),
                        stop=(st == NST - 1 and g == NG - 1))

        def q_chunk(sc):
            csl = slice(sc * SC, (sc + 1) * SC)
            for g in range(NG):
                q_ps = ps_q.tile([P, SC], F32, tag="q_ps")
                for dt in range(ndt):
                    nc.tensor.matmul(
                        q_ps, wq_sb[:, dt, g * P:(g + 1) * P],
                        x_sb[:, dt, csl],
                        start=(dt == 0), stop=(dt == ndt - 1))
                m_q = sb_a.tile([P, SC], BF16, tag="m_q")
                nc.vector.tensor_scalar(m_q, q_ps, 0.0, None, op0=ALU.min)
                nc.scalar.activation(m_q, m_q, AF.Exp)
                nc.vector.scalar_tensor_tensor(
                    qf_sb[:, g, csl], q_ps, 0.0, m_q,
                    op0=ALU.max, op1=ALU.add)

        # KV of the LAST chunk goes last so the boundary extraction (which
        # waits on the kvt/ksum stop matmuls) is never queued behind the
        # final Q-feature DVE work -> no PE gap at the A->B boundary
        kv_chunk(0); q_chunk(0)
        kv_chunk(1); q_chunk(1)
        kv_chunk(2); q_chunk(2)
        q_chunk(3); kv_chunk(3)

    # ---- boundary: masked KV^T / bden extraction + fused M = bkv @ wo ----
    # bkvT[g][cv, ck] = KV^T for head(cv)==head(ck) else 0
    # bden[g][ck', ck] = ksum[ck'] for head(ck')==head(ck) else 0
    for g in range(NG):
        for hb in range(2):
            hsl = slice(hb * 64, (hb + 1) * 64)
            nc.scalar.copy(out=bkvT_sb[hsl, g, hsl], in_=kvt_ps[g][hsl, hsl])
            nc.vector.tensor_copy(
                out=bden_sb[hsl, g, hsl],
                in_=ksum_ps[hsl, g:g + 1].to_broadcast((64, 64)))

    with tc.tile_pool(name="ps_m", bufs=2, space="PSUM") as ps_m:
        for g in range(NG):
            for eh in range(2):
                esl = slice(eh * EH, (eh + 1) * EH)
                m_ps = ps_m.tile([P, EH], F32, tag="m_ps")
                nc.tensor.matmul(m_ps, bkvT_sb[:, g, :], wo_sb[:, g, esl],
                                 start=True, stop=True)
                if eh == 0:
                    nc.scalar.copy(out=m_sb[:, g, esl], in_=m_ps)
                else:
                    nc.vector.tensor_copy(out=m_sb[:, g, esl], in_=m_ps)


def _emit_outproj(nc, sb_b, ps_o, m_sb, out2, qs_pair, sc):
    # two half-chunk staging tiles so each 512KB DMA departs after only 4 of
    # the 8 PSUM->SBUF copies (shorter drain tail on the last chunk)
    for half in range(2):
        o_sb = sb_b.tile([P, STC // 2, D], BF16, tag=f"o_sb{half}")
        for hi in range(STC // 2):
            sti = half * 2 + hi
            for eh in range(2):
                esl = slice(eh * EH, (eh + 1) * EH)
                o_ps = ps_o.tile([P, EH], F32, tag=f"o_ps{eh}")
                for g in range(NG):
                    nc.tensor.matmul(
                        o_ps, qs_pair[g][:, sti * P:(sti + 1) * P],
                        m_sb[:, g, esl], start=(g == 0), stop=(g == NG - 1))
                # PSUM reads must be scalar/vector: split 4/4 per chunk
                if eh == 0:
                    nc.scalar.copy(out=o_sb[:, hi, esl], in_=o_ps)
                else:
                    nc.vector.tensor_copy(out=o_sb[:, hi, esl], in_=o_ps)
        nc.sync.dma_start(
            out=out2[:, sc * STC + half * 2:sc * STC + (half + 1) * 2, :],
            in_=o_sb)


def build_nc(use_biases):
    nc = bacc.Bacc("TRN2", target_bir_lowering=False, debug=False)
    ndt = 9 if use_biases else 8
    t = {}
    t["xA"] = nc.dram_tensor("xA", [STC * P, 8 * P], BF16,
                             kind="ExternalInput")
    t["xB"] = nc.dram_tensor("xB", [(NSC - 1) * P, 8 * SC], BF16,
                             kind="ExternalInput")
    t["wqT"] = nc.dram_tensor("wqT", [P, ndt * CSL], BF16,
                              kind="ExternalInput")
    t["wkvT"] = nc.dram_tensor("wkvT", [P, ndt * 2 * CSL], BF16,
                               kind="ExternalInput")
    t["woT"] = nc.dram_tensor("woT", [P, NG * D], BF16, kind="ExternalInput")
    t["out2"] = nc.dram_tensor("out2", [S, D], BF16, kind="ExternalOutput")

    with tile.TileContext(nc) as tc:
        with ExitStack() as ctx:
            _build_kernel_body(ctx, tc, t, use_biases)
    nc.compile()
    return nc


def _get_nc(use_biases):
    key = ("nc", use_biases)
    if key not in _CACHE:
        _CACHE[key] = build_nc(use_biases)
    return _CACHE[key]


def make_in_maps(x, wq, bq, wk, bk, wv, bv, wo, bo, use_biases=None):
    """Shard the full inputs into the 8 per-core input maps."""
    f = lambda a: np.asarray(a, dtype=np.float32)
    x, wq, bq, wk, bk = f(x), f(wq), f(bq), f(wk), f(bk)
    wv, bv, wo, bo = f(wv), f(bv), f(wo), f(bo)
    if use_biases is None:
        # bo is applied host-side in unshard(); bq/bk/bv need the in-GEMM path
        use_biases = any(np.any(b) for b in (bq, bk, bv))
    bf = lambda a: np.ascontiguousarray(a).astype(NPBF)
    ndt = 9 if use_biases else 8

    def tiled_w(wT):
        # [ndt*128, c] -> [128, ndt*c]: per-partition contiguous weight rows
        c = wT.shape[1]
        return wT.reshape(ndt, P, c).transpose(1, 0, 2).reshape(P, ndt * c)

    xparts = {}
    for b in range(BATCH):
        xt = np.ascontiguousarray(x[b].T)            # [D, S]
        v4 = xt.reshape(8, P, NST, P)
        xA = v4[:, :, 0:STC, :].transpose(2, 1, 0, 3).reshape(STC * P, 8 * P)
        v5 = xt.reshape(8, P, NSC, SC)
        xB = (v5[:, :, 1:NSC, :].transpose(2, 1, 0, 3)
              .reshape((NSC - 1) * P, 8 * SC))
        xparts[b] = (bf(xA), bf(xB))

    in_maps = []
    for cid in range(8):
        b, hg = divmod(cid, 4)
        hs = slice(hg * CSL, (hg + 1) * CSL)
        wkvT = np.concatenate([wk[hs, :].T, wv[hs, :].T], axis=1)
        wqT = wq[hs, :].T
        if use_biases:
            # bias row at row D (multiplied by the on-chip ones row), zero
            # padding to the 9*128 augmented contraction size
            wkvT = np.concatenate(
                [wkvT, np.concatenate([bk[hs], bv[hs]])[None, :],
                 np.zeros((P - 1, 2 * CSL), np.float32)], axis=0)
            wqT = np.concatenate(
                [wqT, bq[hs][None, :], np.zeros((P - 1, CSL), np.float32)],
                axis=0)
        woT = wo[:, hs].T.reshape(NG, P, D).transpose(1, 0, 2).reshape(P, -1)
        m = {
            "xA": xparts[b][0],
            "xB": xparts[b][1],
            "wqT": bf(tiled_w(wqT)),
            "wkvT": bf(tiled_w(wkvT)),
            "woT": bf(woT),
        }
        in_maps.append(m)
    return in_maps, use_biases


def unshard(results, bo=None):
    """Sum head-group partials per batch (tensor-parallel unshard)."""
    out = np.zeros((BATCH, S, D), np.float32)
    for cid in range(8):
        b = cid // 4
        out[b] += np.asarray(results[cid]["out2"]).astype(np.float32)
    if bo is not None:
        bo = np.asarray(bo, np.float32)
        if np.any(bo):
            out += bo[None, None, :]
    return out


def kernel(x, wq, bq, wk, bk, wv, bv, wo, bo):
    in_maps, use_biases = make_in_maps(x, wq, bq, wk, bk, wv, bv, wo, bo)
    nc = _get_nc(use_biases)
    res = bass_utils.run_bass_kernel_spmd(nc, in_maps, core_ids=list(range(8)))
    return unshard(res.results, bo=bo)
